# revision 1
# baseline (speedup 1.0000x reference)
"""Trainium2 Bass kernel for a single-step DecoderRNN (embed+ReLU -> LSTM cell
-> vocab projection -> log_softmax), sharded across 8 NeuronCores.

Sharding strategy (hardcoded):
  * The hidden dimension H=2048 is sharded 8-ways (256 units per core).
  * Each core holds the rows of W_ih/W_hh for its 4x256 gate slice
    (pre-transposed on host into matmul-ready lhsT layout), computes its
    slice of the LSTM cell state, and then computes partial logits
    logits_c = W_out[:, slice].T-dot-h_new[slice] over the FULL vocab.
  * Host sums the 8 partial-logit vectors, adds b_out, applies log_softmax,
    and concatenates the h/c shards.  The embedding row is gathered on host
    (only one row of emb is ever read) and broadcast to all cores.
"""

import functools
import sys

sys.path.insert(0, "/opt/trn_rl_repo")

import numpy as np

import concourse.bass as bass  # noqa: F401  (bass types used via bacc/tile)
import concourse.mybir as mybir
import concourse.tile as tile
from concourse import bacc
from concourse.bass_utils import run_bass_kernel_spmd

F32 = mybir.dt.float32
AF = mybir.ActivationFunctionType

H = 2048
V = 50257
NCORES = 8
HSH = H // NCORES  # 256 hidden units per core
KV = HSH // 128  # 2 k-chunks for the logits matmul
VPAD = 50304  # 393 * 128
MCH = VPAD // 128  # 393 output column-chunks of 128 logits
# Column widths for streaming W_out tiles (large first, small last so the
# PE tail after the final DMA is short).  Sums to VPAD.
WOUT_TILES = [5120] * 9 + [2048, 1152, 1024]
assert sum(WOUT_TILES) == VPAD

# Layout of the packed per-core "small" input tensor [128, 42]:
#   cols 0:16   x_raw (embedding row, pre-relu), chunk c at col c
#   cols 16:32  h0 (full hidden state)
#   cols 32:34  c0 shard (256 values)
#   cols 34:42  b_ih+b_hh shard (1024 values, gate-major i,f,g,o)
SMALL_COLS = 42


@functools.lru_cache(maxsize=1)
def _build():
    nc = bacc.Bacc("TRN2", target_bir_lowering=False, debug=False)
    sm_d = nc.dram_tensor("small", [128, SMALL_COLS], F32, kind="ExternalInput")
    wg_d = nc.dram_tensor("wg", [128, 32, 1024], F32, kind="ExternalInput")
    wo_d = nc.dram_tensor("wout", [128, KV, VPAD], F32, kind="ExternalInput")
    plog_d = nc.dram_tensor("plog", [128, MCH], F32, kind="ExternalOutput")
    hc_d = nc.dram_tensor("hc", [128, 4], F32, kind="ExternalOutput")

    with tile.TileContext(nc) as tc:
        with (
            tc.tile_pool(name="smalls", bufs=1) as smp,
            tc.tile_pool(name="wgp", bufs=3) as wgp,
            tc.tile_pool(name="wop", bufs=3) as wop,
            tc.tile_pool(name="pp", bufs=8, space="PSUM") as pp,
            tc.tile_pool(name="outp", bufs=1) as outp,
        ):
            sm = smp.tile([128, SMALL_COLS], F32)
            nc.sync.dma_start(out=sm, in_=sm_d.ap())
            x_r = smp.tile([128, 16], F32)
            nc.scalar.activation(x_r, sm[:, 0:16], AF.Relu)

            # ---- gates = W_ih_sh @ relu(x) + W_hh_sh @ h0  (1024 values) ----
            # 8 output chunks of 128, each accumulating in its own PSUM bank
            # over 32 contraction chunks (16 for W_ih on x, 16 for W_hh on h).
            pg = []
            for m in range(8):
                pgt = pp.tile([128, 1], F32, tag="pb", name=f"pg{m}")
                pg.append(pgt)
            for t4 in range(8):
                wgt = wgp.tile([128, 4, 1024], F32, tag="wg", name=f"wgt{t4}")
                nc.sync.dma_start(out=wgt, in_=wg_d.ap()[:, t4 * 4 : t4 * 4 + 4, :])
                for kk in range(4):
                    kc = t4 * 4 + kk
                    # x chunks live in x_r cols 0..15; h chunks in sm cols 16..31.
                    rhs = x_r[:, kc : kc + 1] if kc < 16 else sm[:, kc : kc + 1]
                    for m in range(8):
                        nc.tensor.matmul(
                            pg[m][:, :],
                            lhsT=wgt[:, kk, m * 128 : (m + 1) * 128],
                            rhs=rhs,
                            start=(kc == 0),
                            stop=(kc == 31),
                        )

            # ---- nonlinearities + cell update (256 units, [128, 2] tiles) ----
            # chunk m holds gate rows m*128..m*128+127: m 0-1 -> i, 2-3 -> f,
            # 4-5 -> g (tanh), 6-7 -> o.  Bias is fused into the activation.
            g_sb = outp.tile([128, 8], F32)
            for m in range(8):
                func = AF.Tanh if m in (4, 5) else AF.Sigmoid
                nc.scalar.activation(
                    g_sb[:, m : m + 1],
                    pg[m][:, :],
                    func,
                    bias=sm[:, 34 + m : 35 + m],
                )
            hc_t = outp.tile([128, 4], F32)
            fc = outp.tile([128, 2], F32)
            ig = outp.tile([128, 2], F32)
            tct = outp.tile([128, 2], F32)
            nc.vector.tensor_mul(fc, g_sb[:, 2:4], sm[:, 32:34])  # f * c0
            nc.vector.tensor_mul(ig, g_sb[:, 0:2], g_sb[:, 4:6])  # i * g
            nc.vector.tensor_add(hc_t[:, 2:4], fc, ig)  # c_new
            nc.scalar.activation(tct, hc_t[:, 2:4], AF.Tanh)
            nc.vector.tensor_mul(hc_t[:, 0:2], g_sb[:, 6:8], tct)  # h_new
            nc.sync.dma_start(out=hc_d.ap(), in_=hc_t)

            # ---- partial logits over the full vocab ----
            # plog[p, col] = sum_k W_out[col*128+p, csl+k] * h_new[csl+k]
            pl = pp.tile([128, 512], F32, tag="pb", name="plpsum")
            col0 = 0
            for ncols in WOUT_TILES:
                wot = wop.tile([128, KV, ncols], F32, tag="wo", name="wot")
                nc.gpsimd.dma_start(out=wot, in_=wo_d.ap()[:, :, col0 : col0 + ncols])
                for mm in range(ncols // 128):
                    col = col0 // 128 + mm
                    nc.tensor.matmul(
                        pl[:, col : col + 1],
                        lhsT=wot[:, 0, mm * 128 : (mm + 1) * 128],
                        rhs=hc_t[:, 0:1],
                        start=True,
                        stop=False,
                    )
                    nc.tensor.matmul(
                        pl[:, col : col + 1],
                        lhsT=wot[:, 1, mm * 128 : (mm + 1) * 128],
                        rhs=hc_t[:, 1:2],
                        start=False,
                        stop=True,
                    )
                col0 += ncols
            pl_sb = outp.tile([128, MCH], F32)
            nc.vector.tensor_copy(pl_sb, pl[:, 0:MCH])
            nc.sync.dma_start(out=plog_d.ap(), in_=pl_sb)

    nc.compile()
    return nc


def _prep_in_maps(inputs: dict) -> list[dict]:
    token = int(np.asarray(inputs["input"]).reshape(-1)[0])
    x_raw = np.asarray(inputs["emb"][token], dtype=np.float32).reshape(H)
    h0 = np.asarray(inputs["h0"], dtype=np.float32).reshape(H)
    c0 = np.asarray(inputs["c0"], dtype=np.float32).reshape(H)
    W_ih = np.asarray(inputs["W_ih"], dtype=np.float32)
    W_hh = np.asarray(inputs["W_hh"], dtype=np.float32)
    bsum = (
        np.asarray(inputs["b_ih"], dtype=np.float32)
        + np.asarray(inputs["b_hh"], dtype=np.float32)
    ).reshape(4, H)
    W_out = np.asarray(inputs["W_out"], dtype=np.float32)

    x_t = x_raw.reshape(16, 128).T
    h_t = h0.reshape(16, 128).T
    W_ih4 = W_ih.reshape(4, H, H)
    W_hh4 = W_hh.reshape(4, H, H)

    in_maps = []
    for c in range(NCORES):
        sl = slice(c * HSH, (c + 1) * HSH)
        small = np.empty((128, SMALL_COLS), dtype=np.float32)
        small[:, 0:16] = x_t
        small[:, 16:32] = h_t
        small[:, 32:34] = c0[sl].reshape(2, 128).T
        small[:, 34:42] = bsum[:, sl].reshape(8, 128).T

        # wg[p, kc, m]: kc 0..15 -> W_ih_sh.T chunks, 16..31 -> W_hh_sh.T.
        wg = np.empty((128, 32, 1024), dtype=np.float32)
        wih_sh = W_ih4[:, sl, :].reshape(1024, H)  # [4*256, 2048]
        whh_sh = W_hh4[:, sl, :].reshape(1024, H)
        wg[:, 0:16, :] = wih_sh.T.reshape(16, 128, 1024).transpose(1, 0, 2)
        wg[:, 16:32, :] = whh_sh.T.reshape(16, 128, 1024).transpose(1, 0, 2)

        # wout[p, kk, v] = W_out[v, c*256 + kk*128 + p], zero-padded in v.
        wo = np.zeros((128, KV, VPAD), dtype=np.float32)
        ws = W_out[:, sl].T  # [256, V]
        wo[:, :, 0:V] = ws.reshape(KV, 128, V).transpose(1, 0, 2)

        in_maps.append(
            {"small": small, "wg": np.ascontiguousarray(wg), "wout": wo}
        )
    return in_maps


def _postprocess(results: list[dict], inputs: dict):
    b_out = np.asarray(inputs["b_out"], dtype=np.float64).reshape(V)
    logits = np.zeros(V, dtype=np.float64)
    h_new = np.empty(H, dtype=np.float32)
    c_new = np.empty(H, dtype=np.float32)
    for c in range(NCORES):
        plog = results[c]["plog"]  # [128, MCH]
        logits += plog.T.reshape(VPAD)[:V].astype(np.float64)
        hc = results[c]["hc"]  # [128, 4]
        sl = slice(c * HSH, (c + 1) * HSH)
        h_new[sl] = hc[:, 0:2].T.reshape(HSH)
        c_new[sl] = hc[:, 2:4].T.reshape(HSH)
    logits += b_out
    m = logits.max()
    logp = (logits - (m + np.log(np.exp(logits - m).sum()))).astype(np.float32)
    return (
        logp.reshape(1, V),
        h_new.reshape(1, 1, H),
        c_new.reshape(1, 1, H),
    )


def _run(inputs: dict, **spmd_kwargs):
    nc = _build()
    in_maps = _prep_in_maps(inputs)
    res = run_bass_kernel_spmd(nc, in_maps, list(range(NCORES)), **spmd_kwargs)
    return _postprocess(res.results, inputs), res


def kernel(**inputs):
    out, _ = _run(inputs)
    return out
